# revision 27
# baseline (speedup 1.0000x reference)
"""Trainium2 Bass kernel for nn_AttentionLayer_83545703842160.

Single-head attention over spatial tokens, per batch element:
  t = x[b].reshape(C, H*W).T            # [N, C], N=4096, C=64
  q,k,v = t@W{q,k,v}.T + b{q,k,v}
  out   = softmax(q@k.T / sqrt(C)) @ v  # -> [C, N] -> [C, H, W]

Sharding: data-parallel over batch B=8 across the 8 NeuronCores (one
batch element per core). Each core holds the full (tiny) QKV weights.

Per-core kernel. Projections run in float32r (full-precision inputs);
the two big attention matmuls stream in bf16 (measured ~30% faster than
f32r on HW; fp32 PSUM accumulation, HW rel err ~3e-3 vs the fp32 ref):
  - xt [65, 4096] SBUF f32r: x[b] in [C, N] layout + a host-appended ones
    row so biases fold into the contraction (K = 65).
  - qT,kT [64, 4096] bf16 = W{q,k}_ext @ xt (PE), cast PSUM->SBUF by DVE.
  - v_sb [128, 32, 66] bf16 token-major v with a ones column (col 64, for
    the softmax denominator) and a zero pad column (f32r even-count rule).
  - main loop over 8 query superblocks (S=512) x 32 key tiles (128):
      MM1: sT[m-tile 128, S] = kT-slice.T @ qT-slice   (PSUM fp32)
      ACT: exp(0.125 * sT) PSUM->SBUF in [128, <=1536] chunks (3 m-tiles
           per ACTIVATE amortize the ~220-cycle per-instruction bubble)
      MM2: acc[66, S] += v_ext[m].T @ pT   (PSUM accumulate; row 64
           accumulates the softmax denominator via the ones column)
      tail: recip(rowsum) -> gpsimd partition_broadcast -> DVE multiply
            -> DMA out y[:, block]
  Stage-2 (MM2+tail) lags stage-1 by one superblock and its MM2 groups
  are interleaved 1:1 with the next superblock's MM1 groups, so the PE
  never has a burst that starves the exp stream.  The deep pt pool
  carries a superblock of exp'd scores between the stages.  Projection
  producers (k chunks, v tiles) trickle through superblock 0's groups.
  PSUM: scores ping-pong 2x3 banks + acc/projection pool 2x1 = 8 banks.

The ScalarE (ACT) engine is the theoretical bottleneck: softmax must
exp N^2 = 16.7M elements/core at 1 elem/lane/cycle @ 1.2 GHz (~110 us
floor, ~127 us with per-instruction bubbles).  PE (~119 us of matmul),
DVE (~32 us), and DMA (~2.5 MiB) largely hide underneath it.
Measured per-body on HW (reps-loop delta method): ~0.2 ms.
"""

import numpy as np
from contextlib import ExitStack

import concourse.bacc as bacc
import concourse.bass as bass
import concourse.mybir as mybir
import concourse.tile as tile
from concourse.bass import MemorySpace
from concourse.bass_utils import run_bass_kernel_spmd

C = 64          # channels
N = 4096        # tokens (64*64 spatial)
B = 8           # batch == number of cores
S = 512         # query superblock
MT = 128        # keys per m-tile
NMT = N // MT   # 32 m-tiles
WCOLS = 2 * C + C + 2   # packed weight tensor: [wq | wk | wv_ext]
FP32 = mybir.dt.float32
F32R = mybir.dt.float32r
BF16 = mybir.dt.bfloat16
EXP = mybir.ActivationFunctionType.Exp
# m-tiles per ACTIVATE chunk: 3 tiles -> [128, 1536] = 3 PSUM banks.
# The short group leads: its cheap exp is covered by the previous (long)
# group's exp while the PE refills the first slot after a superblock switch.
GROUPS = [2] + [3] * 10
assert sum(GROUPS) == NMT


def _build_kernel(tc, ctx, x_d, w_d, y_d, reps=1):
    if reps > 1:
        # timing harness: repeat the whole body in a HW loop so kernel time
        # dominates dispatch overhead in wallclock measurements
        with tc.For_i(0, reps, 1):
            _build_body(tc, ctx, x_d, w_d, y_d)
    else:
        _build_body(tc, ctx, x_d, w_d, y_d)


def _build_body(tc, ctx, x_d, w_d, y_d):
    nc = tc.nc

    sb = ctx.enter_context(tc.tile_pool(name="sb", bufs=1))
    pt_pool = ctx.enter_context(tc.tile_pool(name="pt", bufs=20))
    osb_pool = ctx.enter_context(tc.tile_pool(name="osb", bufs=2))
    nrm_pool = ctx.enter_context(tc.tile_pool(name="nrm", bufs=2))
    sc_psum = ctx.enter_context(
        tc.tile_pool(name="scp", bufs=2, space=MemorySpace.PSUM))
    ac_psum = ctx.enter_context(
        tc.tile_pool(name="acp", bufs=2, space=MemorySpace.PSUM))

    xt = sb.tile([C + 1, N], F32R)
    w_sb = sb.tile([C + 1, WCOLS], F32R)
    qt = sb.tile([C, N], BF16)
    kt = sb.tile([C, N], BF16)
    v_sb = sb.tile([MT, NMT, C + 2], BF16)

    wq = w_sb[:, 0:C]
    wk = w_sb[:, C:2 * C]
    wv = w_sb[:, 2 * C:WCOLS]

    # DMA issue costs ~0.65us per descriptor per issuing engine: the first
    # x chunk and the (single, packed) weight tensor go on sync's queue so
    # the first scores group unblocks ASAP; remaining x chunks stream in on
    # gpsimd's queue in parallel.
    nc.sync.dma_start(xt[:, 0:S], x_d[:, 0:S])
    nc.sync.dma_start(w_sb[:], w_d)
    for j in range(1, N // S):
        nc.gpsimd.dma_start(xt[:, j * S:(j + 1) * S],
                            x_d[:, j * S:(j + 1) * S])

    # Projection producers, emitted piecemeal so they interleave with the
    # exp stream instead of forming a serial head phase.
    def emit_qk(w_slice, dst, j):
        p = ac_psum.tile([C, S], FP32, tag="ps1")
        nc.tensor.matmul(p[:], w_slice, xt[:, j * S:(j + 1) * S],
                         start=True, stop=True)
        nc.vector.tensor_copy(dst[:, j * S:(j + 1) * S], p[:])

    def emit_v(m):
        p = ac_psum.tile([MT, C + 2], FP32, tag="ps1")
        nc.tensor.matmul(p[:], xt[:, m * MT:(m + 1) * MT], wv,
                         start=True, stop=True)
        nc.vector.tensor_copy(v_sb[:, m, :], p[:])

    # head: only what the very first scores group needs
    emit_qk(wq, qt, 0)
    emit_qk(wk, kt, 0)

    def emit_stage2_group(acc, pts, gi):
        m = sum(GROUPS[:gi])
        for j in range(GROUPS[gi]):
            mm = m + j
            nc.tensor.matmul(
                acc[:], v_sb[:, mm, :], pts[gi][:, j * S:(j + 1) * S],
                start=(mm == 0), stop=(mm == NMT - 1))

    def emit_tail(acc, s):
        # normalize: y[:, block] = acc[0:64] / acc[64] (denominator row)
        rs = nrm_pool.tile([1, S], FP32, tag="rs")
        nc.vector.tensor_copy(rs[:], acc[C:C + 1, :])
        rr = nrm_pool.tile([1, S], FP32, tag="rr")
        nc.vector.reciprocal(rr[:], rs[:])
        bc = nrm_pool.tile([C, S], FP32, tag="bc")
        nc.gpsimd.partition_broadcast(bc[:], rr[:], channels=C)
        ob = osb_pool.tile([C, S], FP32, tag="ob")
        nc.vector.tensor_mul(ob[:], acc[0:C, :], bc[:])
        nc.sync.dma_start(y_d[:, s * S:(s + 1) * S], ob[:])

    # v-tile production schedule: ~2 tiles per group through nsb0's groups
    # 1-10 (v[0..19]) and nsb1's groups 0-5 (v[20..31]); each tile lands at
    # least one full group before the stage-2 MM2 that consumes it.
    v_sched = {(0, gi): list(range(2 * (gi - 1), 2 * gi)) for gi in range(1, 11)}
    for gi in range(6):
        v_sched[(1, gi)] = list(range(20 + 2 * gi, min(22 + 2 * gi, NMT)))

    prev = None
    for s in range(N // S):
        qs = qt[:, s * S:(s + 1) * S]
        if s < N // S - 1:
            emit_qk(wq, qt, s + 1)  # q chunk for the NEXT superblock
        if prev is not None:
            acc = ac_psum.tile([C + 2, S], FP32, tag="ps1")
        else:
            acc = None
        pts = []
        m = 0
        for gi, gs in enumerate(GROUPS):
            if s == 0 and gi <= 6:
                # k chunk gi+1 lands one group before any MM1 needs it
                emit_qk(wk, kt, gi + 1)
            for mv in v_sched.get((s, gi), ()):
                emit_v(mv)
            sc = sc_psum.tile([MT, gs * S], FP32, tag="sc")
            # interleave this group's MM1s 1:1 with the previous
            # superblock's stage-2 MM2s so accumulating matmuls into the
            # same PSUM bank are never back-to-back on the PE
            mprev = sum(GROUPS[:gi])
            for j in range(gs):
                nc.tensor.matmul(
                    sc[:, j * S:(j + 1) * S],
                    kt[:, (m + j) * MT:(m + j + 1) * MT], qs,
                    start=True, stop=True)
                if prev is not None and j < GROUPS[gi]:
                    mm = mprev + j
                    nc.tensor.matmul(
                        acc[:], v_sb[:, mm, :],
                        prev[1][gi][:, j * S:(j + 1) * S],
                        start=(mm == 0), stop=(mm == NMT - 1))
            pt = pt_pool.tile([MT, gs * S], BF16, tag="pt")
            nc.scalar.activation(pt[:], sc[:], EXP, scale=0.125)
            pts.append(pt)
            m += gs
        if prev is not None:
            emit_tail(acc, prev[0])
        prev = (s, pts)
    acc = ac_psum.tile([C + 2, S], FP32, tag="ps1")
    for gi in range(len(GROUPS)):
        emit_stage2_group(acc, prev[1], gi)
    emit_tail(acc, prev[0])


_NC_CACHE = {}


def _get_nc(reps=1):
    if reps not in _NC_CACHE:
        nc = bacc.Bacc("TRN2", target_bir_lowering=False, debug=False,
                       enable_asserts=False)
        x_d = nc.dram_tensor("x", [C + 1, N], F32R, kind="ExternalInput").ap()
        w_d = nc.dram_tensor("w", [C + 1, WCOLS], F32R,
                             kind="ExternalInput").ap()
        y_d = nc.dram_tensor("y", [C, N], FP32, kind="ExternalOutput").ap()
        with tile.TileContext(nc) as tc:
            with ExitStack() as ctx:
                _build_kernel(tc, ctx, x_d, w_d, y_d, reps=reps)
        nc.compile()
        _NC_CACHE[reps] = nc
    return _NC_CACHE[reps]


def _host_weights(Wq, bq, Wk, bk, Wv, bv):
    w = np.zeros((C + 1, WCOLS), np.float32)
    w[:C, 0:C] = Wq.T
    w[C, 0:C] = bq
    w[:C, C:2 * C] = Wk.T
    w[C, C:2 * C] = bk
    w[:C, 2 * C:3 * C] = Wv.T
    w[C, 2 * C:3 * C] = bv
    w[C, 3 * C] = 1.0  # ones column of v_ext; col 3C+1 stays zero padding
    return w


def _host_x(x_b):
    return np.ascontiguousarray(
        np.concatenate([x_b.reshape(C, N), np.ones((1, N), np.float32)],
                       axis=0))


def _run(inputs, reps=1, **spmd_kwargs):
    x = np.ascontiguousarray(np.asarray(inputs["x"], np.float32))
    w = _host_weights(
        np.asarray(inputs["Wq"], np.float32),
        np.asarray(inputs["bq"], np.float32),
        np.asarray(inputs["Wk"], np.float32),
        np.asarray(inputs["bk"], np.float32),
        np.asarray(inputs["Wv"], np.float32),
        np.asarray(inputs["bv"], np.float32))
    nc = _get_nc(reps)
    in_maps = [{"x": _host_x(x[b]), "w": w} for b in range(B)]
    res = run_bass_kernel_spmd(nc, in_maps, core_ids=list(range(B)),
                               **spmd_kwargs)
    out = np.stack([res.results[b]["y"].reshape(C, 64, 64)
                    for b in range(B)], axis=0)
    return out, res


def kernel(**inputs):
    out, _ = _run(inputs)
    return out


# revision 28
# speedup vs baseline: 1.0175x; 1.0175x over previous
"""Trainium2 Bass kernel for nn_AttentionLayer_83545703842160.

Single-head attention over spatial tokens, per batch element:
  t = x[b].reshape(C, H*W).T            # [N, C], N=4096, C=64
  q,k,v = t@W{q,k,v}.T + b{q,k,v}
  out   = softmax(q@k.T / sqrt(C)) @ v  # -> [C, N] -> [C, H, W]

Sharding: data-parallel over batch B=8 across the 8 NeuronCores (one
batch element per core). Each core holds the full (tiny) QKV weights.

Per-core kernel. Projections run in float32r (full-precision inputs);
the two big attention matmuls stream in bf16 (measured ~30% faster than
f32r on HW; fp32 PSUM accumulation, HW rel err ~3e-3 vs the fp32 ref):
  - xt [65, 4096] SBUF f32r: x[b] in [C, N] layout + a host-appended ones
    row so biases fold into the contraction (K = 65).
  - qT,kT [64, 4096] bf16 = W{q,k}_ext @ xt (PE), cast PSUM->SBUF by DVE.
  - v_sb [128, 32, 66] bf16 token-major v with a ones column (col 64, for
    the softmax denominator) and a zero pad column (f32r even-count rule).
  - main loop over 8 query superblocks (S=512) x 32 key tiles (128):
      MM1: sT[m-tile 128, S] = kT-slice.T @ qT-slice   (PSUM fp32)
      ACT: exp(0.125 * sT) PSUM->SBUF in [128, <=1536] chunks (3 m-tiles
           per ACTIVATE amortize the ~220-cycle per-instruction bubble)
      MM2: acc[66, S] += v_ext[m].T @ pT   (PSUM accumulate; row 64
           accumulates the softmax denominator via the ones column)
      tail: recip(rowsum) -> gpsimd partition_broadcast -> DVE multiply
            -> DMA out y[:, block]
  Stage-2 (MM2+tail) lags stage-1 by one superblock and its MM2 groups
  are interleaved 1:1 with the next superblock's MM1 groups, so the PE
  never has a burst that starves the exp stream.  The deep pt pool
  carries a superblock of exp'd scores between the stages.  Projection
  producers (k chunks, v tiles) trickle through superblock 0's groups.
  PSUM: scores ping-pong 2x3 banks + acc/projection pool 2x1 = 8 banks.

The ScalarE (ACT) engine is the theoretical bottleneck: softmax must
exp N^2 = 16.7M elements/core at 1 elem/lane/cycle @ 1.2 GHz (~110 us
floor, ~127 us with per-instruction bubbles).  PE (~119 us of matmul),
DVE (~32 us), and DMA (~2.5 MiB) largely hide underneath it.
Measured per-body on HW (reps-loop delta method): ~0.2 ms.
"""

import numpy as np
from contextlib import ExitStack

import concourse.bacc as bacc
import concourse.bass as bass
import concourse.mybir as mybir
import concourse.tile as tile
from concourse.bass import MemorySpace
from concourse.bass_utils import run_bass_kernel_spmd

C = 64          # channels
N = 4096        # tokens (64*64 spatial)
B = 8           # batch == number of cores
S = 512         # query superblock
MT = 128        # keys per m-tile
NMT = N // MT   # 32 m-tiles
WCOLS = 2 * C + C + 2   # packed weight tensor: [wq | wk | wv_ext]
FP32 = mybir.dt.float32
F32R = mybir.dt.float32r
BF16 = mybir.dt.bfloat16
EXP = mybir.ActivationFunctionType.Exp
# m-tiles per ACTIVATE chunk: 3 tiles -> [128, 1536] = 3 PSUM banks.
# The short group leads: its cheap exp is covered by the previous (long)
# group's exp while the PE refills the first slot after a superblock switch.
GROUPS = [2] + [3] * 10
assert sum(GROUPS) == NMT


def _build_kernel(tc, ctx, x_d, w_d, y_d, reps=1):
    if reps > 1:
        # timing harness: repeat the whole body in a HW loop so kernel time
        # dominates dispatch overhead in wallclock measurements
        engines = (mybir.EngineType.PE, mybir.EngineType.Activation,
                   mybir.EngineType.DVE, mybir.EngineType.Pool,
                   mybir.EngineType.SP)
        with tc.For_i(0, reps, 1, hint_engines=engines):
            _build_body(tc, ctx, x_d, w_d, y_d)
    else:
        _build_body(tc, ctx, x_d, w_d, y_d)


def _build_body(tc, ctx, x_d, w_d, y_d):
    nc = tc.nc

    sb = ctx.enter_context(tc.tile_pool(name="sb", bufs=1))
    pt_pool = ctx.enter_context(tc.tile_pool(name="pt", bufs=20))
    osb_pool = ctx.enter_context(tc.tile_pool(name="osb", bufs=2))
    nrm_pool = ctx.enter_context(tc.tile_pool(name="nrm", bufs=2))
    sc_psum = ctx.enter_context(
        tc.tile_pool(name="scp", bufs=2, space=MemorySpace.PSUM))
    ac_psum = ctx.enter_context(
        tc.tile_pool(name="acp", bufs=2, space=MemorySpace.PSUM))

    xt = sb.tile([C + 1, N], F32R)
    w_sb = sb.tile([C + 1, WCOLS], F32R)
    qt = sb.tile([C, N], BF16)
    kt = sb.tile([C, N], BF16)
    v_sb = sb.tile([MT, NMT, C + 2], BF16)

    wq = w_sb[:, 0:C]
    wk = w_sb[:, C:2 * C]
    wv = w_sb[:, 2 * C:WCOLS]

    # DMA issue costs ~0.65us per descriptor per issuing engine: the first
    # x chunk and the (single, packed) weight tensor go on sync's queue so
    # the first scores group unblocks ASAP; remaining x chunks stream in on
    # gpsimd's queue in parallel.
    nc.sync.dma_start(xt[:, 0:S], x_d[:, 0:S])
    nc.sync.dma_start(w_sb[:], w_d)
    for j in range(1, N // S):
        nc.gpsimd.dma_start(xt[:, j * S:(j + 1) * S],
                            x_d[:, j * S:(j + 1) * S])

    # Projection producers, emitted piecemeal so they interleave with the
    # exp stream instead of forming a serial head phase.
    def emit_qk(w_slice, dst, j):
        p = ac_psum.tile([C, S], FP32, tag="ps1")
        nc.tensor.matmul(p[:], w_slice, xt[:, j * S:(j + 1) * S],
                         start=True, stop=True)
        nc.vector.tensor_copy(dst[:, j * S:(j + 1) * S], p[:])

    def emit_v(m):
        p = ac_psum.tile([MT, C + 2], FP32, tag="ps1")
        nc.tensor.matmul(p[:], xt[:, m * MT:(m + 1) * MT], wv,
                         start=True, stop=True)
        nc.vector.tensor_copy(v_sb[:, m, :], p[:])

    # head: only what the very first scores group needs
    emit_qk(wq, qt, 0)
    emit_qk(wk, kt, 0)

    def emit_stage2_group(acc, pts, gi):
        m = sum(GROUPS[:gi])
        for j in range(GROUPS[gi]):
            mm = m + j
            nc.tensor.matmul(
                acc[:], v_sb[:, mm, :], pts[gi][:, j * S:(j + 1) * S],
                start=(mm == 0), stop=(mm == NMT - 1))

    def emit_tail(acc, s):
        # normalize: y[:, block] = acc[0:64] / acc[64] (denominator row)
        rs = nrm_pool.tile([1, S], FP32, tag="rs")
        nc.vector.tensor_copy(rs[:], acc[C:C + 1, :])
        rr = nrm_pool.tile([1, S], FP32, tag="rr")
        nc.vector.reciprocal(rr[:], rs[:])
        bc = nrm_pool.tile([C, S], FP32, tag="bc")
        nc.gpsimd.partition_broadcast(bc[:], rr[:], channels=C)
        ob = osb_pool.tile([C, S], FP32, tag="ob")
        nc.vector.tensor_mul(ob[:], acc[0:C, :], bc[:])
        nc.sync.dma_start(y_d[:, s * S:(s + 1) * S], ob[:])

    # v-tile production schedule: ~2 tiles per group through nsb0's groups
    # 1-10 (v[0..19]) and nsb1's groups 0-5 (v[20..31]); each tile lands at
    # least one full group before the stage-2 MM2 that consumes it.
    v_sched = {(0, gi): list(range(2 * (gi - 1), 2 * gi)) for gi in range(1, 11)}
    for gi in range(6):
        v_sched[(1, gi)] = list(range(20 + 2 * gi, min(22 + 2 * gi, NMT)))

    prev = None
    for s in range(N // S):
        qs = qt[:, s * S:(s + 1) * S]
        if s < N // S - 1:
            emit_qk(wq, qt, s + 1)  # q chunk for the NEXT superblock
        if prev is not None:
            acc = ac_psum.tile([C + 2, S], FP32, tag="ps1")
        else:
            acc = None
        pts = []
        m = 0
        for gi, gs in enumerate(GROUPS):
            if s == 0 and gi <= 6:
                # k chunk gi+1 lands one group before any MM1 needs it
                emit_qk(wk, kt, gi + 1)
            for mv in v_sched.get((s, gi), ()):
                emit_v(mv)
            sc = sc_psum.tile([MT, gs * S], FP32, tag="sc")
            # interleave this group's MM1s 1:1 with the previous
            # superblock's stage-2 MM2s so accumulating matmuls into the
            # same PSUM bank are never back-to-back on the PE
            mprev = sum(GROUPS[:gi])
            for j in range(gs):
                nc.tensor.matmul(
                    sc[:, j * S:(j + 1) * S],
                    kt[:, (m + j) * MT:(m + j + 1) * MT], qs,
                    start=True, stop=True)
                if prev is not None and j < GROUPS[gi]:
                    mm = mprev + j
                    nc.tensor.matmul(
                        acc[:], v_sb[:, mm, :],
                        prev[1][gi][:, j * S:(j + 1) * S],
                        start=(mm == 0), stop=(mm == NMT - 1))
            pt = pt_pool.tile([MT, gs * S], BF16, tag="pt")
            nc.scalar.activation(pt[:], sc[:], EXP, scale=0.125)
            pts.append(pt)
            m += gs
        if prev is not None:
            emit_tail(acc, prev[0])
        prev = (s, pts)
    acc = ac_psum.tile([C + 2, S], FP32, tag="ps1")
    for gi in range(len(GROUPS)):
        emit_stage2_group(acc, prev[1], gi)
    emit_tail(acc, prev[0])


_NC_CACHE = {}


def _get_nc(reps=1):
    if reps not in _NC_CACHE:
        nc = bacc.Bacc("TRN2", target_bir_lowering=False, debug=False,
                       enable_asserts=False)
        x_d = nc.dram_tensor("x", [C + 1, N], F32R, kind="ExternalInput").ap()
        w_d = nc.dram_tensor("w", [C + 1, WCOLS], F32R,
                             kind="ExternalInput").ap()
        y_d = nc.dram_tensor("y", [C, N], FP32, kind="ExternalOutput").ap()
        with tile.TileContext(nc) as tc:
            with ExitStack() as ctx:
                _build_kernel(tc, ctx, x_d, w_d, y_d, reps=reps)
        nc.compile()
        _NC_CACHE[reps] = nc
    return _NC_CACHE[reps]


def _host_weights(Wq, bq, Wk, bk, Wv, bv):
    w = np.zeros((C + 1, WCOLS), np.float32)
    w[:C, 0:C] = Wq.T
    w[C, 0:C] = bq
    w[:C, C:2 * C] = Wk.T
    w[C, C:2 * C] = bk
    w[:C, 2 * C:3 * C] = Wv.T
    w[C, 2 * C:3 * C] = bv
    w[C, 3 * C] = 1.0  # ones column of v_ext; col 3C+1 stays zero padding
    return w


def _host_x(x_b):
    return np.ascontiguousarray(
        np.concatenate([x_b.reshape(C, N), np.ones((1, N), np.float32)],
                       axis=0))


def _run(inputs, reps=1, **spmd_kwargs):
    x = np.ascontiguousarray(np.asarray(inputs["x"], np.float32))
    w = _host_weights(
        np.asarray(inputs["Wq"], np.float32),
        np.asarray(inputs["bq"], np.float32),
        np.asarray(inputs["Wk"], np.float32),
        np.asarray(inputs["bk"], np.float32),
        np.asarray(inputs["Wv"], np.float32),
        np.asarray(inputs["bv"], np.float32))
    nc = _get_nc(reps)
    in_maps = [{"x": _host_x(x[b]), "w": w} for b in range(B)]
    res = run_bass_kernel_spmd(nc, in_maps, core_ids=list(range(B)),
                               **spmd_kwargs)
    out = np.stack([res.results[b]["y"].reshape(C, 64, 64)
                    for b in range(B)], axis=0)
    return out, res


def kernel(**inputs):
    out, _ = _run(inputs)
    return out
